# revision 40
# baseline (speedup 1.0000x reference)
"""Trainium2 Bass kernel for nn_Attention1 (squared-difference attention).

Math (per batch b):
    q  = t1 @ Wq,  k = t1 @ Wk,  p = t2 @ Wp,  v = t1 @ Wv     (per head h: 64 dims)
    raw   = q_h @ (k_h - p_h)^T                                 [n, n]
    logit = raw^2 / sqrt(512)
    attn  = softmax(logit, axis=-1)
    out   = concat_h(attn @ v_h) @ Wo + bo

Sharding: 16 (b, h) pairs over 8 cores -> 2 heads + 1 batch per core.
Each core computes its 2 heads end-to-end including its partial
out-projection (row-sharded Wo); host sums the 4 partials per batch and
adds bo (the "all-reduce after to_out" of the hint, done at unshard time).

Device dataflow per core (n=2048, d=64, 2 heads):
  Projections: x1t/x2t loaded as [128,2048] row-chunks (gpsimd queue,
  wq first so q starts early); PE computes qT/kpT/vT = W^T @ x^T (kp
  uses -Wp folded on host); vT is evicted to bf16 and DMA-transposed to
  the natural [j,d] layout vnat for attn@v.
  Dots: row-packed K=64 matmul pairs (head0 rows 0-63, head1 rows
  64-127) into separate [128,1024] PSUM tiles.
  Softmax: a registered custom DVE op (body sq(Src0)*C1, accum=min)
  fuses square+scale+row-max in ONE pass from PSUM:
  nsq = -(raw^2)*SCALE -> sbuf, negm = running min (= -rowmax logits).
  ACT then does s = exp(-nsq + negm) in bf16 with the rowsum riding the
  activation accumulator (so ACT runs exp only).
  s is DMA-transposed with a per-partition-contiguous dest layout
  st[h][128, c, t, 128]; attn@v col-packs the two heads
  (tile_position=(0,h*64) pairs run concurrently); the group tail
  (attn@v, 1/r via DMA-broadcast reciprocals on the gpsimd queue,
  out-projection vs the Wo row slice) is software-pipelined across the
  next group's chunks; DMA out fp32, host sums partials + bo.
"""

import os
import sys

import numpy as np

for _p in ("/opt/trn_rl_repo",):
    if _p not in sys.path:
        sys.path.insert(0, _p)

import concourse.bass as bass
import concourse.mybir as mybir
import concourse.tile as tile
from concourse.bass_utils import run_bass_kernel_spmd
from concourse.vector_clock import ScopedClock, VectorClock

import concourse.dve_ops as dvo
from concourse.dve_spec import Spec, Src0, C0, C1, sq, minn, lower
from concourse.dve_spec import _has_src1
from concourse.dve_uop import DveOpSpec


def _patched_drain_and_barrier(self, tick_clock, wait_clock):
    """Kernel-tail drain emitting one wait per proc.

    The stock tail puts every proc's sem wait on a single Drain; this
    container's walrus ("Too many sync wait commands") only accepts one
    wait per instruction, so split the clock per proc and use the NRT
    pseudo sync barrier in place of the direct EVSEM butterfly.
    """
    nc = self.nc
    gc = tick_clock.global_clock
    nprocs = len(gc)
    for i in range(nprocs):
        t = gc[i]
        if t <= 0:
            continue
        vc = VectorClock([0] * nprocs)
        vc.require_at_least(i, t)
        d = nc.sync.drain()
        wait_clock.add_sem_waits(d.ins, ScopedClock({None: vc}))
    nc.all_engine_barrier()
    popped = nc._tile_sem_poison_stack.pop()
    assert popped is self._sem_poison
    nc.clear_and_free_semaphores(list(self.sems.allocated().values()))
    nc.all_engine_barrier()


tile.TileContext._drain_and_barrier = _patched_drain_and_barrier


def _split_multi_waits(nc):
    """This container's walrus accepts one sync wait per instruction;
    move extra waits onto EventSemaphore instructions inserted before."""
    import bass_rust

    n = 0
    for fn in nc.m.functions:
        for bb in fn.blocks:
            il = bb.instructions
            out = []
            for inst in il:
                si = inst.sync_info
                waits = list(si.on_wait) if (si is not None and si.on_wait) else []
                if len(waits) > 1:
                    for w in waits[:-1]:
                        n += 1
                        ev = mybir.InstEventSemaphore(
                            name=f"SW-{n}-{inst.name}",
                            engine=inst.engine,
                            debug=inst.debug,
                            sync_info=bass_rust.SyncInfo(on_wait=[w], on_update=[]),
                        )
                        out.append(ev)
                    si.on_wait = [waits[-1]]
                out.append(inst)
            il[:] = out
    return n


FP32 = mybir.dt.float32
BF16 = mybir.dt.bfloat16
AF = mybir.ActivationFunctionType
ALU = mybir.AluOpType
FP32R = mybir.dt.float32r


def _r(ap):
    return ap.bitcast(FP32R)


DIM = 512
HEADS = 8
DH = 64
N = 2048
B = 2
SCALE = float(DIM) ** -0.5
NCORES = 8

NI = N // 128          # 16 i-chunks of 128
NG = 4                 # groups of 4 i-chunks (512 rows)
CPG = NI // NG         # chunks per group = 4
NJ = N // 128          # 16 j-chunks
JW = 1024              # dots psum tile width (j); 2 halves per row


# ---------------- custom DVE op: nsq = sq(x)*c1, accum_out = min ------------
def _nsq_ref(in0, in1, s0, s1, imm2):
    b = (in0.astype(np.float32) ** 2 * s1).astype(np.float32)
    acc = np.minimum(
        np.asarray(s0, np.float32),
        b.reshape(b.shape[0], -1).min(axis=-1, keepdims=True),
    ).astype(np.float32)
    return b, acc


def _register_nsq_op():
    name = "NSQ_MIN_REDUCE_ANT"
    for o in dvo.OPS:
        if o.name == name:
            return o
    spec = Spec(
        body=sq(Src0) * C1,
        accum=minn,
        accum_init=C0,
        reference=_nsq_ref,
    )
    row = max(dvo._SUB_OPCODE_FOR_NAME.values()) + 1
    assert row < 0x20
    dvo._SUB_OPCODE_FOR_NAME[name] = row
    shas = {}
    for ver in ("v3", "v4"):
        tmp = DveOpSpec(
            name=name, opcode=row, uops=lower(spec, ver=ver),
            rd1_en=_has_src1(spec),
        )
        shas[ver] = tmp.sha(ver)
    op = dvo.DveOp(name, spec, subdim=False, uops_sha=shas)
    dvo.OPS.append(op)
    dvo.CUSTOM_DVE_SPECS[name] = spec
    return op


NSQ_OP = _register_nsq_op()


def build_bass(split_waits: bool = True) -> bass.Bass:
    nc = bass.Bass()

    x1t = nc.dram_tensor("x1t", [DIM, N], FP32, kind="ExternalInput")
    x2t = nc.dram_tensor("x2t", [DIM, N], FP32, kind="ExternalInput")
    wq = nc.dram_tensor("wq", [DIM, 128], FP32, kind="ExternalInput")
    wk = nc.dram_tensor("wk", [DIM, 128], FP32, kind="ExternalInput")
    wpn = nc.dram_tensor("wpn", [DIM, 128], FP32, kind="ExternalInput")
    wv = nc.dram_tensor("wv", [DIM, 128], FP32, kind="ExternalInput")
    wo = nc.dram_tensor("wo", [128, DIM], FP32, kind="ExternalInput")
    out = nc.dram_tensor("out", [N, DIM], FP32, kind="ExternalOutput")
    rscr = nc.dram_tensor("rscr", [2, N], FP32)

    with tile.TileContext(nc) as tc:
        with (
            tc.tile_pool(name="persist", bufs=1) as persist,
            tc.tile_pool(name="work", bufs=3) as work,
        ):
            # weights (loads interleaved with x below: wq first so q-proj
            # can start as soon as x1 lands)
            def load_w(dram, name):
                t = persist.tile([128, 4, 128], FP32R, tag=name)
                nc.gpsimd.dma_start(
                    t[:], _r(dram[:].rearrange("(kc p) m -> p kc m", p=128))
                )
                return t

            wq_sb = load_w(wq, "wq")

            # PE warm-up: ~5us of matmuls gated on the wq DMA, timed to end
            # as the first projection MM issues, so HAM reaches K=8/8 and
            # the projections run at 2.4GHz instead of the cold 1.2GHz.
            with tc.tile_pool(name="warm", bufs=1, space="PSUM") as wp:
                pw = wp.tile([128, 512], FP32, tag="pw", name="pw")
                wq_flat = wq_sb[:].rearrange("p a b -> p (a b)")
                for _ in range(12):
                    nc.tensor.matmul(
                        pw[0:64, :], wq_sb[:, 0, 0:64], wq_flat,
                        start=True, stop=True,
                    )

            qT = persist.tile([128, N], FP32R, tag="qT", name="qT")
            kpT = persist.tile([128, N], FP32R, tag="kpT", name="kpT")
            vnat = persist.tile([128, NJ, 128], BF16, tag="vnat", name="vnat")
            vT_bf = persist.tile([128, N], BF16, tag="vT", name="vT")
            rcol = [
                persist.tile([128, NI], FP32, tag=f"rcol{h}", name=f"rcol{h}")
                for h in range(2)
            ]

            # ---------------- projections ----------------
            with tc.tile_pool(name="xp", bufs=1) as xp:
                x1 = []
                x2 = []
                for kc in range(4):
                    t1c = xp.tile([128, N], FP32R, tag=f"x1_{kc}",
                                  name=f"x1_{kc}")
                    nc.gpsimd.dma_start(
                        t1c[:], _r(x1t[kc * 128:(kc + 1) * 128, :])
                    )
                    x1.append(t1c)
                wk_sb = load_w(wk, "wk")
                wpn_sb = load_w(wpn, "wpn")
                for kc in range(4):
                    t2c = xp.tile([128, N], FP32R, tag=f"x2_{kc}",
                                  name=f"x2_{kc}")
                    nc.gpsimd.dma_start(
                        t2c[:], _r(x2t[kc * 128:(kc + 1) * 128, :])
                    )
                    x2.append(t2c)
                wv_sb = load_w(wv, "wv")
                wo_sb = persist.tile([128, DIM], FP32R, tag="wo", name="wo")
                nc.gpsimd.dma_start(wo_sb[:], _r(wo[:]))

                with tc.tile_pool(name="pja", bufs=2, space="PSUM") as pja, \
                        tc.tile_pool(name="pjk", bufs=4, space="PSUM") as pjk:
                    # q: contiguous 4-MM groups per nb
                    for nb in range(4):
                        psq = pja.tile([128, 512], FP32, tag="psq", name="psq")
                        for kc in range(4):
                            nc.tensor.matmul(
                                psq[:], wq_sb[:, kc, :],
                                x1[kc][:, nb * 512:(nb + 1) * 512],
                                start=(kc == 0), stop=(kc == 3),
                            )
                        nc.vector.tensor_copy(
                            qT[:, nb * 512:(nb + 1) * 512], psq[:]
                        )
                    # kp: kc-outer interleaved groups (v1 pattern)
                    psk = [pjk.tile([128, 512], FP32, tag="psk", name="psk")
                           for _ in range(4)]
                    for kc in range(4):
                        for nb in range(4):
                            nc.tensor.matmul(
                                psk[nb][:], wk_sb[:, kc, :],
                                x1[kc][:, nb * 512:(nb + 1) * 512],
                                start=(kc == 0), stop=False,
                            )
                        for nb in range(4):
                            nc.tensor.matmul(
                                psk[nb][:], wpn_sb[:, kc, :],
                                x2[kc][:, nb * 512:(nb + 1) * 512],
                                start=False, stop=(kc == 3),
                            )
                    for nb in range(4):
                        nc.vector.tensor_copy(
                            kpT[:, nb * 512:(nb + 1) * 512], psk[nb][:]
                        )

                with tc.tile_pool(name="pjb", bufs=2, space="PSUM") as pjb:
                    for nb in range(4):
                        psv = pjb.tile([128, 512], FP32, tag="psv", name="psv")
                        for kc in range(4):
                            nc.tensor.matmul(
                                psv[:], wv_sb[:, kc, :],
                                x1[kc][:, nb * 512:(nb + 1) * 512],
                                start=(kc == 0), stop=(kc == 3),
                            )
                        nc.scalar.copy(
                            vT_bf[:, nb * 512:(nb + 1) * 512], psv[:]
                        )

                # v to natural [j, d] layout: vnat[p, t, dd] = vT[dd, t*128+p]
                nc.sync.dma_start_transpose(vnat[:], vT_bf[:])

            # ---------------- attention ----------------
            with (
                tc.tile_pool(name="nsqp", bufs=3) as nsqp,
                tc.tile_pool(name="spool", bufs=6) as spool,
                tc.tile_pool(name="stp", bufs=2) as stp,
                tc.tile_pool(name="psd", bufs=3, space="PSUM") as psd_pool,
                tc.tile_pool(name="pso", bufs=1, space="PSUM") as pso_pool,
                tc.tile_pool(name="pss", bufs=1, space="PSUM") as pss_pool,
            ):
                # deferred group tail: av matmuls split into independent
                # c-half accumulations (half 0 covers chunks 0-1 and can
                # run during the group's OWN chunks 2-3; half 1 spreads
                # over the next group) -> PE stays busy, tail drain halves
                pending = None  # dict(g, st_g, rb, pso)

                def av_half(p, ch, t0, n_t):
                    for t in range(t0, t0 + n_t):
                        for h in range(2):
                            nc.tensor.matmul(
                                p["pso"][h * 64:(h + 1) * 64,
                                         ch * 256:(ch + 1) * 256],
                                vnat[:, t, h * 64:(h + 1) * 64],
                                p["st_g"][h][:, 2 * ch:2 * ch + 2, t, :],
                                start=(t == 0),
                                stop=(t == NJ - 1),
                                tile_position=(0, h * 64),
                                skip_group_check=True,
                            )

                def finish_tail():
                    g = pending["g"]
                    attn_sb = work.tile([128, 512], FP32R, tag="attn",
                                        name="attn")
                    nc.vector.tensor_mul(attn_sb[:], pending["pso"][:],
                                         pending["rb"][:])
                    # out projection for the 4 i-chunks of this group
                    for c in range(CPG):
                        psp = pss_pool.tile([128, 512], FP32, tag="pss",
                                            name="pss")
                        nc.tensor.matmul(
                            psp[:],
                            attn_sb[:, c * 128:(c + 1) * 128],
                            wo_sb[:],
                            start=True,
                            stop=True,
                        )
                        out_sb = work.tile([128, 512], FP32, tag="out",
                                           name="out")
                        if c % 2 == 0:
                            nc.scalar.copy(out_sb[:], psp[:])
                        else:
                            nc.vector.tensor_copy(out_sb[:], psp[:])
                        nc.gpsimd.dma_start(
                            out[(g * CPG + c) * 128:(g * CPG + c + 1) * 128, :],
                            out_sb[:],
                        )

                st_g = None
                cur = None  # current group's record (pso alloc deferred)
                for gcn in range(NI):
                    g, c = divmod(gcn, CPG)
                    if c == 0:
                        st_g = [
                            stp.tile([128, CPG, NJ, 128], BF16, tag=f"st{h}",
                                     name=f"st{h}")
                            for h in range(2)
                        ]
                        cur = {"g": g, "st_g": st_g, "rb": None, "pso": None}
                    i0 = gcn * 128
                    nsq = [
                        nsqp.tile([128, N], FP32, tag=f"nsq{h}", name=f"nsq{h}")
                        for h in range(2)
                    ]
                    negm = [
                        work.tile([128, 1], FP32, tag=f"negm{h}",
                                  name=f"negm{h}")
                        for h in range(2)
                    ]
                    for jh in range(2):
                        psd = [
                            psd_pool.tile([128, JW], FP32, tag="psd",
                                          name="psd")
                            for _ in range(2)
                        ]
                        for jq in range(2):
                            j0 = jh * JW + jq * 512
                            js = slice(jq * 512, (jq + 1) * 512)
                            for h in range(2):
                                hp = h * 64
                                nc.tensor.matmul(
                                    psd[h][:, js],
                                    qT[hp:hp + 64, i0:i0 + 128],
                                    kpT[hp:hp + 64, j0:j0 + 512],
                                    start=True,
                                    stop=True,
                                )
                        # nsq = -(raw^2)*SCALE, running min -> negm
                        for h in range(2):
                            nc.vector._custom_dve(
                                NSQ_OP,
                                out=nsq[h][:, jh * JW:(jh + 1) * JW],
                                in0=psd[h][:],
                                s0=(3.0e38 if jh == 0 else negm[h][:]),
                                s1=-SCALE,
                                accum_out=negm[h][:],
                            )
                    for h in range(2):
                        s_t = spool.tile([128, N], BF16, tag="s", name="s_t")
                        nc.scalar.activation(
                            s_t[:],
                            nsq[h][:],
                            AF.Exp,
                            bias=negm[h][:],
                            scale=-1.0,
                            accum_out=rcol[h][:, gcn:gcn + 1],
                        )
                        nc.sync.dma_start_transpose(
                            st_g[h][:, c, :, :],
                            s_t[:],
                        )

                    # av schedule: prev group's c23-half over chunks 0-1,
                    # current group's c01-half over its own chunks 2-3
                    if c == 0 and pending is not None:
                        av_half(pending, 1, 0, NJ // 2)
                    elif c == 1 and pending is not None:
                        av_half(pending, 1, NJ // 2, NJ // 2)
                        finish_tail()
                        pending = None
                    elif c == 2:
                        cur["pso"] = pso_pool.tile([128, 512], FP32,
                                                   tag="pso", name="pso")
                        av_half(cur, 0, 0, NJ // 2)
                    elif c == 3:
                        av_half(cur, 0, NJ // 2, NJ // 2)

                    if c == CPG - 1:
                        # reciprocal of r -> DRAM -> broadcast (gpsimd queue,
                        # off the transpose-laden sync queue)
                        rb = work.tile([128, 512], FP32, tag="rb", name="rb")
                        for h in range(2):
                            rrec = work.tile([128, CPG], FP32, tag="rrec",
                                             name="rrec")
                            nc.vector.reciprocal(
                                rrec[:], rcol[h][:, g * CPG:(g + 1) * CPG]
                            )
                            nc.gpsimd.dma_start(
                                rscr[h, g * 512:(g + 1) * 512].rearrange(
                                    "(cc p) -> p cc", p=128
                                ),
                                rrec[:],
                            )
                            nc.gpsimd.dma_start(
                                rb[h * 64:(h + 1) * 64, :],
                                rscr[h, g * 512:(g + 1) * 512]
                                .unsqueeze(0)
                                .broadcast_to((64, 512)),
                            )
                        cur["rb"] = rb
                        pending = cur

                # drain the last group's c23-half and tail
                av_half(pending, 1, 0, NJ)
                finish_tail()

    # populate .instr bytes for InstISA subclasses (InstCustomDveAnt);
    # raw Bass doesn't run this pass -> "ISA wrong length" without it.
    mybir.codegen_inst_isa_subclasses(nc)
    if split_waits:
        _split_multi_waits(nc)
    return nc


_NC = None


def _get_nc():
    global _NC
    if _NC is None:
        _NC = build_bass()
    return _NC


def _shard_inputs(t1, t2, Wq, Wk, Wv, Wp, Wo, bo):
    t1 = np.asarray(t1, np.float32)
    t2 = np.asarray(t2, np.float32)
    x1ts = [np.ascontiguousarray(t1[b].T) for b in range(B)]
    x2ts = [np.ascontiguousarray(t2[b].T) for b in range(B)]
    Wq = np.asarray(Wq, np.float32)
    Wk = np.asarray(Wk, np.float32)
    Wv = np.asarray(Wv, np.float32)
    Wpn = -np.asarray(Wp, np.float32)
    Wo = np.asarray(Wo, np.float32)

    in_maps = []
    for c in range(NCORES):
        b = c // 4
        h0 = (2 * c) % 8
        hs = slice(64 * h0, 64 * h0 + 128)
        in_maps.append(
            {
                "x1t": x1ts[b],
                "x2t": x2ts[b],
                "wq": np.ascontiguousarray(Wq[:, hs]),
                "wk": np.ascontiguousarray(Wk[:, hs]),
                "wpn": np.ascontiguousarray(Wpn[:, hs]),
                "wv": np.ascontiguousarray(Wv[:, hs]),
                "wo": np.ascontiguousarray(Wo[hs, :]),
            }
        )
    return in_maps


def kernel(t1, t2, Wq, Wk, Wv, Wp, Wo, bo, _trace=False):
    nc = _get_nc()
    in_maps = _shard_inputs(t1, t2, Wq, Wk, Wv, Wp, Wo, bo)
    res = run_bass_kernel_spmd(
        nc, in_maps, list(range(NCORES)), trace=_trace,
        tmpdir=os.environ.get("BASS_TMPDIR"),
    )
    parts = [np.asarray(r["out"], np.float32) for r in res.results]
    bo32 = np.asarray(bo, np.float32)
    out = np.zeros((B, N, DIM), np.float32)
    for b in range(B):
        out[b] = (
            parts[4 * b] + parts[4 * b + 1] + parts[4 * b + 2] + parts[4 * b + 3]
            + bo32
        )
    kernel.last_exec_time_ns = res.exec_time_ns
    kernel.last_results = res
    return out


if __name__ == "__main__":
    # quick CoreSim smoke test of one core
    from concourse.bass_interp import CoreSim
    import reference as ref

    inputs = {k: np.asarray(v) for k, v in ref.setup_inputs().items()}
    nc = build_bass(split_waits=False)
    nc.finalize()
    in_maps = _shard_inputs(**inputs)
    core = int(os.environ.get("SMOKE_CORE", "0"))
    sim = CoreSim(nc)
    for k, v in in_maps[core].items():
        sim.tensor(k)[:] = v
    sim.simulate()
    got = np.array(sim.tensor("out"))
    # expected partial for this core (no bo; host adds it)
    t1, t2 = inputs["t1"], inputs["t2"]
    b = core // 4
    h0 = (2 * core) % 8
    acc = np.zeros((N, DIM), np.float32)
    for h in (h0, h0 + 1):
        q = (t1[b] @ inputs["Wq"][:, h * 64:(h + 1) * 64])
        kp = t1[b] @ inputs["Wk"][:, h * 64:(h + 1) * 64] - t2[b] @ inputs["Wp"][:, h * 64:(h + 1) * 64]
        v = t1[b] @ inputs["Wv"][:, h * 64:(h + 1) * 64]
        raw = np.asarray(q @ kp.T, np.float32)
        logits = SCALE * raw * raw
        s = np.exp(logits - logits.max(axis=1, keepdims=True))
        o = (s @ v) / s.sum(axis=1, keepdims=True)
        acc += o @ inputs["Wo"][h * 64:(h + 1) * 64, :]
    err = np.abs(got - acc)
    denom = np.abs(acc).max()
    print("core", core, "absmax err:", err.max(), "rel:", err.max() / denom)
